# revision 8
# baseline (speedup 1.0000x reference)
"""Conditional Instance Norm (CIN) kernel for Trainium2, data-parallel over batch.

Reference semantics (per batch sample b, channel c):
    gamma_mix = style_weights @ gammas          # [B, C]
    beta_mix  = style_weights @ betas           # [B, C]
    y[b,c]    = gamma_mix[b,c] * (x[b,c] - mean) * rsqrt(var + eps) + beta_mix[b,c]
with mean/var over the spatial dims of x[b,c] (biased var).

Strategy: one batch sample per NeuronCore (B=8 samples, 8 cores).  The
correctness gate is rel_err < 2e-2, so x is quantized to bf16 on the host and
y is produced in bf16 (~4e-3 worst-case error): HBM traffic halves to
32 MiB in + 32 MiB out per core, an ~187 us floor at the ~358 GB/s
per-core HBM limit.

Per core, x is [C=256, HW=65536] bf16.  Channels are processed in tiles of
G channels; each channel's HW elements are laid out over Q=128/G partitions,
so a tile is a dense [128, F=HW/Q] SBUF block read from HBM exactly once and
written exactly once.

Engine balance at bf16 pace (DMA period ~11.7 us per G=16 tile):
  ACT: Square w/ accum_out (sum of squares)  ~6.8 us
       + Sqrt(var+eps) for the previous tile + apply of an Fa-column chunk
  DVE: reduce_sum (per-partition sums), fused scale*x+bias tensor_scalar on
       the remaining F-Fa columns, and the tiny per-channel stats chain
  PE:  two tiny matmuls (fold Q partitions -> per-channel stats; broadcast
       per-channel scale/bias -> per-partition)
The loop is software-pipelined with lag 2 (stats for tile t, rstd/scale for
tile t-1, apply+store for tile t-2) so no engine queue ever blocks another.

PE Matmult instructions only tolerate a single sync-wait, so every matmul
operand is funneled through a DVE-produced tile.
"""

import sys

for _p in ("/opt/trn_rl_repo",):
    if _p not in sys.path:
        sys.path.insert(0, _p)

from contextlib import ExitStack

import numpy as np

import concourse.bacc as bacc
import concourse.tile as tile
from concourse import mybir
from concourse.bass_utils import run_bass_kernel_spmd

EPS = 1e-5

# Full problem dims (hardcoded per harness contract).
B, C, H, W = 8, 256, 256, 256
S = 16
HW = H * W
N_CORES = 8
P = 128  # SBUF partitions

AF = mybir.ActivationFunctionType
ALU = mybir.AluOpType
f32 = mybir.dt.float32
bf16 = mybir.dt.bfloat16


def _const_layout(C_, S_, G):
    """Column offsets of the packed constants tensor: g4 | e4 | gammas | betas | sw."""
    o_g4 = 0
    o_e4 = o_g4 + G
    o_gam = o_e4 + P
    o_bet = o_gam + C_
    o_sw = o_bet + C_
    ncols = o_sw + 1
    return o_g4, o_e4, o_gam, o_bet, o_sw, ncols


DEFAULTS = dict(G=16, xt_bufs=8, act_frac=0.0, io_dtype="bf16")


def build_cin_program(
    C_=C,
    HW_=HW,
    S_=S,
    G=DEFAULTS["G"],  # channels per tile
    xt_bufs=DEFAULTS["xt_bufs"],
    act_frac=DEFAULTS["act_frac"],  # fraction of each tile applied on ACT
    io_dtype=DEFAULTS["io_dtype"],  # "bf16" or "f32"
    reps=1,  # repeat the main loop (for slope-based benchmarking)
):
    """Trace the per-core CIN program.  Returns the Bass module."""
    Q = P // G  # partitions per channel
    F = HW_ // Q  # free elems per partition
    NT = C_ // G  # number of tiles
    assert P % G == 0 and HW_ % Q == 0 and C_ % G == 0

    xdt = bf16 if io_dtype == "bf16" else f32
    # ACT applies columns [0:Fa], DVE applies [Fa:F]; keep 4B-aligned chunks
    Fa = int(F * act_frac) // 2 * 2

    o_g4, o_e4, o_gam, o_bet, o_sw, NCOLS = _const_layout(C_, S_, G)

    nc = bacc.Bacc(trn_type="TRN2")

    x_d = nc.dram_tensor("x", [C_ * Q, F], xdt, kind="ExternalInput")
    consts_d = nc.dram_tensor("consts", [P, NCOLS], f32, kind="ExternalInput")
    y_d = nc.dram_tensor("y", [C_ * Q, F], xdt, kind="ExternalOutput")

    NI = NT * reps  # total tile iterations
    LAG = 2  # apply/store runs LAG iterations behind load/stats

    with tile.TileContext(nc) as tc, ExitStack() as ctx:
        xpool = ctx.enter_context(tc.tile_pool(name="xt", bufs=xt_bufs))
        sqpool = ctx.enter_context(tc.tile_pool(name="sq", bufs=1))
        dmypool = ctx.enter_context(tc.tile_pool(name="dmy", bufs=1))
        ppool = ctx.enter_context(tc.tile_pool(name="part", bufs=3))
        stpool = ctx.enter_context(tc.tile_pool(name="st", bufs=3))
        sbpool = ctx.enter_context(tc.tile_pool(name="sb", bufs=4))
        singles = ctx.enter_context(tc.tile_pool(name="singles", bufs=1))
        ch_ps = ctx.enter_context(tc.tile_pool(name="chps", bufs=2, space="PSUM"))
        bc_ps = ctx.enter_context(tc.tile_pool(name="bcps", bufs=2, space="PSUM"))
        gb_psp = ctx.enter_context(tc.tile_pool(name="gbps", bufs=1, space="PSUM"))

        # ---- constants: one DMA + one DVE funnel copy ----
        consts_sb = singles.tile([P, NCOLS], f32)
        nc.gpsimd.dma_start(out=consts_sb[:], in_=consts_d[:])
        consts_f = singles.tile([P, NCOLS], f32)
        nc.vector.tensor_copy(consts_f[:], consts_sb[:])

        g4_f = consts_f[:, o_g4 : o_g4 + G]  # [128, G] selector, 1/HW entries
        e4_f = consts_f[0:G, o_e4 : o_e4 + P]  # [G, 128] expander, 0/1 entries
        sw_f = consts_f[0:S_, o_sw : o_sw + 1]  # [S, 1]

        eps_sb = singles.tile([G, 1], f32)
        nc.vector.memset(eps_sb[:], EPS)

        # gb_all[:, t, 0] = gamma_mix for tile t's channels, [:, t, 1] = beta_mix
        gb_ps = gb_psp.tile([G, NT, 2], f32)
        gb_all = singles.tile([G, NT, 2], f32)
        for t in range(NT):
            gam_t = consts_f[0:S_, o_gam + G * t : o_gam + G * (t + 1)]
            bet_t = consts_f[0:S_, o_bet + G * t : o_bet + G * (t + 1)]
            nc.tensor.matmul(gb_ps[:, t, 0:1], gam_t, sw_f, start=True, stop=True)
            nc.tensor.matmul(gb_ps[:, t, 1:2], bet_t, sw_f, start=True, stop=True)
        nc.vector.tensor_copy(gb_all[:], gb_ps[:])

        # ---- software-pipelined main loop ----
        # iteration i: load+front-stats for tile i, rstd/scale/bias for tile
        # i-1, apply+store for tile i-LAG.
        xts = {}  # live x tiles
        sts = {}  # live stats tiles (cols: 0=mean 1=exsq 2=tmp 3=var 4=scale 5=bias 6=std 7=rstd)
        sbs = {}  # live broadcast (scale,bias) tiles

        for i in range(NI + LAG):
            t = i if i < NI else -1
            u = i - 1 if 0 <= i - 1 < NI else -1  # stats-tail tile
            v = i - LAG if 0 <= i - LAG < NI else -1  # apply/store tile

            if t >= 0:
                td = t % NT  # DRAM tile index
                xt = xts[t] = xpool.tile([P, F], xdt, name="xt")
                nc.sync.dma_start(out=xt[:], in_=x_d[P * td : P * (td + 1), :])
                part = ppool.tile([P, 2], f32)
                # per-partition sum via tensor_scalar's accumulator (4x DVE
                # mode; InstTensorReduce only runs at 1 elem/cycle)
                dmy = dmypool.tile([P, F], bf16)
                nc.vector.tensor_scalar(
                    out=dmy[:], in0=xt[:], scalar1=1.0, scalar2=None,
                    op0=ALU.mult, op1=ALU.add, accum_out=part[:, 0:1],
                )
                sq = sqpool.tile([P, F], bf16)
                nc.scalar.activation(
                    out=sq[:], in_=xt[:], func=AF.Square, accum_out=part[:, 1:2]
                )

            if u >= 0:
                # sqrt(var+eps) early on ACT so DVE's reciprocal never waits
                stu = sts[u]
                nc.scalar.activation(
                    out=stu[:, 6:7], in_=stu[:, 3:4], func=AF.Sqrt, bias=eps_sb[:]
                )

            if v >= 0:
                # y = scale * x + bias, in place; ACT does cols [0:Fa] (none
                # by default), DVE the rest, each followed by its own store.
                xv, sbv = xts.pop(v), sbs.pop(v)
                vd = v % NT
                if Fa > 0:
                    nc.scalar.activation(
                        out=xv[:, 0:Fa], in_=xv[:, 0:Fa], func=AF.Identity,
                        bias=sbv[:, 1:2], scale=sbv[:, 0:1],
                    )
                    nc.scalar.dma_start(
                        out=y_d[P * vd : P * (vd + 1), 0:Fa], in_=xv[:, 0:Fa]
                    )
                nc.vector.tensor_scalar(
                    out=xv[:, Fa:F], in0=xv[:, Fa:F],
                    scalar1=sbv[:, 0:1], scalar2=sbv[:, 1:2],
                    op0=ALU.mult, op1=ALU.add,
                )
                nc.gpsimd.dma_start(out=y_d[P * vd : P * (vd + 1), Fa:F], in_=xv[:, Fa:F])

            if t >= 0:
                # funnel (sum, sumsq) through DVE so the PE matmul needs one wait
                part2 = ppool.tile([P, 2], f32, tag="part2")
                nc.vector.tensor_copy(part2[:], part[:])
                # fold Q partitions -> per-channel (mean, E[x^2])
                ch = ch_ps.tile([G, 2], f32)
                nc.tensor.matmul(ch[:], g4_f, part2[:], start=True, stop=True)
                st = sts[t] = stpool.tile([G, 8], f32, name="st")
                nc.vector.tensor_copy(st[:, 0:2], ch[:])
                nc.vector.tensor_mul(st[:, 2:3], st[:, 0:1], st[:, 0:1])
                nc.vector.tensor_sub(st[:, 3:4], st[:, 1:2], st[:, 2:3])  # var

            if u >= 0:
                stu = sts.pop(u)
                ud = u % NT
                nc.vector.reciprocal(stu[:, 7:8], stu[:, 6:7])
                nc.vector.tensor_mul(stu[:, 4:5], stu[:, 7:8], gb_all[:, ud, 0:1])
                nc.vector.tensor_mul(stu[:, 2:3], stu[:, 0:1], stu[:, 4:5])
                nc.vector.tensor_sub(stu[:, 5:6], gb_all[:, ud, 1:2], stu[:, 2:3])
                # broadcast per-channel (scale, bias) back to Q partitions each
                bc = bc_ps.tile([P, 2], f32)
                nc.tensor.matmul(bc[:], e4_f, stu[:, 4:6], start=True, stop=True)
                sb2 = sbs[u] = sbpool.tile([P, 2], f32, name="sb2")
                nc.vector.tensor_copy(sb2[:], bc[:])

    nc.compile()
    return nc


def make_consts(C_=C, HW_=HW, S_=S, G=DEFAULTS["G"], gammas=None, betas=None, sw=None):
    """Host-side packed constants tensor [128, NCOLS]."""
    Q = P // G
    o_g4, o_e4, o_gam, o_bet, o_sw, NCOLS = _const_layout(C_, S_, G)
    consts = np.zeros((P, NCOLS), np.float32)
    consts[np.arange(P), o_g4 + np.arange(P) // Q] = 1.0 / HW_
    consts[np.arange(P) // Q, o_e4 + np.arange(P)] = 1.0
    consts[0:S_, o_gam : o_gam + C_] = gammas
    consts[0:S_, o_bet : o_bet + C_] = betas
    consts[0:S_, o_sw] = sw
    return consts


def io_np_dtype(io_dtype=DEFAULTS["io_dtype"]):
    return mybir.dt.np(bf16 if io_dtype == "bf16" else f32)


def prep_x(x, G=DEFAULTS["G"], io_dtype=DEFAULTS["io_dtype"], B_=B, C_=C, HW_=HW):
    """Reshape full x [B,C,H,W] to per-core [C*Q, F] layout and cast to IO dtype."""
    Q = P // G
    return np.ascontiguousarray(
        np.asarray(x).reshape(B_, C_ * Q, HW_ // Q).astype(io_np_dtype(io_dtype))
    )


_CACHE = {}


def _get_nc():
    if "nc" not in _CACHE:
        _CACHE["nc"] = build_cin_program()
    return _CACHE["nc"]


def kernel(x, style_weights, gammas, betas, _trace=False):
    x = np.asarray(x, dtype=np.float32)
    style_weights = np.ascontiguousarray(np.asarray(style_weights, dtype=np.float32))
    gammas = np.ascontiguousarray(np.asarray(gammas, dtype=np.float32))
    betas = np.ascontiguousarray(np.asarray(betas, dtype=np.float32))

    G = DEFAULTS["G"]
    nc = _get_nc()

    xr = prep_x(x, G)
    in_maps = [
        {
            "x": xr[i],
            "consts": make_consts(C, HW, S, G, gammas, betas, style_weights[i]),
        }
        for i in range(N_CORES)
    ]
    res = run_bass_kernel_spmd(
        nc, in_maps, core_ids=list(range(N_CORES)), trace=_trace
    )
    y = np.stack(
        [
            res.results[i]["y"].astype(np.float32).reshape(C, H, W)
            for i in range(N_CORES)
        ],
        axis=0,
    )
    if _trace:
        return y, res
    return y


# revision 10
# speedup vs baseline: 1.0831x; 1.0831x over previous
"""Conditional Instance Norm (CIN) kernel for Trainium2, data-parallel over batch.

Reference semantics (per batch sample b, channel c):
    gamma_mix = style_weights @ gammas          # [B, C]
    beta_mix  = style_weights @ betas           # [B, C]
    y[b,c]    = gamma_mix[b,c] * (x[b,c] - mean) * rsqrt(var + eps) + beta_mix[b,c]
with mean/var over the spatial dims of x[b,c] (biased var).

Strategy: one batch sample per NeuronCore (B=8 samples, 8 cores).  The
correctness gate is rel_err < 2e-2, so x is quantized to bf16 on the host and
y is produced in bf16 (~4e-3 worst-case error): HBM traffic halves to
32 MiB in + 32 MiB out per core, an ~187 us floor at the ~358 GB/s
per-core HBM limit.

Per core, x is [C=256, HW=65536] bf16.  Channels are processed in tiles of
G channels; each channel's HW elements are laid out over Q=128/G partitions,
so a tile is a dense [128, F=HW/Q] SBUF block read from HBM exactly once and
written exactly once.

Engine balance at bf16 pace (DMA period ~11.7 us per G=16 tile):
  ACT: Square w/ accum_out (sum of squares)  ~6.8 us
       + Sqrt(var+eps) for the previous tile + apply of an Fa-column chunk
  DVE: reduce_sum (per-partition sums), fused scale*x+bias tensor_scalar on
       the remaining F-Fa columns, and the tiny per-channel stats chain
  PE:  two tiny matmuls (fold Q partitions -> per-channel stats; broadcast
       per-channel scale/bias -> per-partition)
The loop is software-pipelined with lag 2 (stats for tile t, rstd/scale for
tile t-1, apply+store for tile t-2) so no engine queue ever blocks another.

PE Matmult instructions only tolerate a single sync-wait, so every matmul
operand is funneled through a DVE-produced tile.
"""

import sys

for _p in ("/opt/trn_rl_repo",):
    if _p not in sys.path:
        sys.path.insert(0, _p)

from contextlib import ExitStack

import numpy as np

import concourse.bacc as bacc
import concourse.tile as tile
from concourse import mybir
from concourse.bass_utils import run_bass_kernel_spmd

EPS = 1e-5

# Full problem dims (hardcoded per harness contract).
B, C, H, W = 8, 256, 256, 256
S = 16
HW = H * W
N_CORES = 8
P = 128  # SBUF partitions

AF = mybir.ActivationFunctionType
ALU = mybir.AluOpType
f32 = mybir.dt.float32
bf16 = mybir.dt.bfloat16


def _const_layout(C_, S_, G):
    """Column offsets of the packed constants tensor: g4 | e4 | gammas | betas | sw."""
    o_g4 = 0
    o_e4 = o_g4 + G
    o_gam = o_e4 + P
    o_bet = o_gam + C_
    o_sw = o_bet + C_
    ncols = o_sw + 1
    return o_g4, o_e4, o_gam, o_bet, o_sw, ncols


DEFAULTS = dict(G=16, xt_bufs=8, act_frac=0.0, io_dtype="bf16", pair=1)


def build_cin_program(
    C_=C,
    HW_=HW,
    S_=S,
    G=DEFAULTS["G"],  # channels per tile
    xt_bufs=DEFAULTS["xt_bufs"],
    act_frac=DEFAULTS["act_frac"],  # fraction of each tile applied on ACT
    io_dtype=DEFAULTS["io_dtype"],  # "bf16" or "f32"
    pair=DEFAULTS["pair"],  # tiles per DMA transfer (1 or 2)
    reps=1,  # repeat the main loop (for slope-based benchmarking)
):
    """Trace the per-core CIN program.  Returns the Bass module."""
    Q = P // G  # partitions per channel
    F = HW_ // Q  # free elems per partition
    NT = C_ // G  # number of tiles
    assert P % G == 0 and HW_ % Q == 0 and C_ % G == 0
    assert pair in (1, 2) and NT % pair == 0

    xdt = bf16 if io_dtype == "bf16" else f32
    # ACT applies columns [0:Fa], DVE applies [Fa:F]; keep 4B-aligned chunks
    Fa = int(F * act_frac) // 2 * 2

    o_g4, o_e4, o_gam, o_bet, o_sw, NCOLS = _const_layout(C_, S_, G)

    nc = bacc.Bacc(trn_type="TRN2")

    # pair=2 packs two consecutive tiles side by side in the free dim (host
    # does the repack) so each load/store is one DMA of 2 tiles.
    x_d = nc.dram_tensor("x", [C_ * Q // pair, pair * F], xdt, kind="ExternalInput")
    consts_d = nc.dram_tensor("consts", [P, NCOLS], f32, kind="ExternalInput")
    y_d = nc.dram_tensor("y", [C_ * Q // pair, pair * F], xdt, kind="ExternalOutput")

    NI = NT * reps  # total tile iterations
    LAG = 2  # apply/store runs LAG iterations behind load/stats

    with tile.TileContext(nc) as tc, ExitStack() as ctx:
        xpool = ctx.enter_context(tc.tile_pool(name="xt", bufs=xt_bufs))
        sqpool = ctx.enter_context(tc.tile_pool(name="sq", bufs=1))
        dmypool = ctx.enter_context(tc.tile_pool(name="dmy", bufs=1))
        ppool = ctx.enter_context(tc.tile_pool(name="part", bufs=3))
        stpool = ctx.enter_context(tc.tile_pool(name="st", bufs=3))
        sbpool = ctx.enter_context(tc.tile_pool(name="sb", bufs=4))
        singles = ctx.enter_context(tc.tile_pool(name="singles", bufs=1))
        ch_ps = ctx.enter_context(tc.tile_pool(name="chps", bufs=2, space="PSUM"))
        bc_ps = ctx.enter_context(tc.tile_pool(name="bcps", bufs=2, space="PSUM"))
        gb_psp = ctx.enter_context(tc.tile_pool(name="gbps", bufs=1, space="PSUM"))

        # ---- constants: one DMA + one DVE funnel copy ----
        consts_sb = singles.tile([P, NCOLS], f32)
        nc.gpsimd.dma_start(out=consts_sb[:], in_=consts_d[:])
        consts_f = singles.tile([P, NCOLS], f32)
        nc.vector.tensor_copy(consts_f[:], consts_sb[:])

        g4_f = consts_f[:, o_g4 : o_g4 + G]  # [128, G] selector, 1/HW entries
        e4_f = consts_f[0:G, o_e4 : o_e4 + P]  # [G, 128] expander, 0/1 entries
        sw_f = consts_f[0:S_, o_sw : o_sw + 1]  # [S, 1]

        eps_sb = singles.tile([G, 1], f32)
        nc.vector.memset(eps_sb[:], EPS)

        # gb_all[:, t, 0] = gamma_mix for tile t's channels, [:, t, 1] = beta_mix
        gb_ps = gb_psp.tile([G, NT, 2], f32)
        gb_all = singles.tile([G, NT, 2], f32)
        for t in range(NT):
            gam_t = consts_f[0:S_, o_gam + G * t : o_gam + G * (t + 1)]
            bet_t = consts_f[0:S_, o_bet + G * t : o_bet + G * (t + 1)]
            nc.tensor.matmul(gb_ps[:, t, 0:1], gam_t, sw_f, start=True, stop=True)
            nc.tensor.matmul(gb_ps[:, t, 1:2], bet_t, sw_f, start=True, stop=True)
        nc.vector.tensor_copy(gb_all[:], gb_ps[:])

        # ---- software-pipelined main loop ----
        # iteration i: load+front-stats for tile i, rstd/scale/bias for tile
        # i-1, apply+store for tile i-LAG.
        xts = {}  # live x tiles
        sts = {}  # live stats tiles (cols: 0=mean 1=exsq 2=tmp 3=var 4=scale 5=bias 6=std 7=rstd)
        sbs = {}  # live broadcast (scale,bias) tiles

        for i in range(NI + LAG):
            t = i if i < NI else -1
            u = i - 1 if 0 <= i - 1 < NI else -1  # stats-tail tile
            v = i - LAG if 0 <= i - LAG < NI else -1  # apply/store tile

            if t >= 0:
                td = t % NT  # DRAM tile index
                xt = xts[t] = xpool.tile([P, F], xdt, name="xt")
                nc.sync.dma_start(out=xt[:], in_=x_d[P * td : P * (td + 1), :])
                part = ppool.tile([P, 2], f32)
                # per-partition sum via tensor_scalar's accumulator (4x DVE
                # mode; InstTensorReduce only runs at 1 elem/cycle)
                dmy = dmypool.tile([P, F], bf16)
                nc.vector.tensor_scalar(
                    out=dmy[:], in0=xt[:], scalar1=1.0, scalar2=None,
                    op0=ALU.mult, op1=ALU.add, accum_out=part[:, 0:1],
                )
                sq = sqpool.tile([P, F], bf16)
                nc.scalar.activation(
                    out=sq[:], in_=xt[:], func=AF.Square, accum_out=part[:, 1:2]
                )

            if u >= 0:
                # sqrt(var+eps) early on ACT so DVE's reciprocal never waits
                stu = sts[u]
                nc.scalar.activation(
                    out=stu[:, 6:7], in_=stu[:, 3:4], func=AF.Sqrt, bias=eps_sb[:]
                )

            if v >= 0:
                # y = scale * x + bias, in place; ACT does cols [0:Fa] (none
                # by default), DVE the rest, each followed by its own store.
                xv, sbv = xts.pop(v), sbs.pop(v)
                vd = v % NT
                if Fa > 0:
                    nc.scalar.activation(
                        out=xv[:, 0:Fa], in_=xv[:, 0:Fa], func=AF.Identity,
                        bias=sbv[:, 1:2], scale=sbv[:, 0:1],
                    )
                    nc.scalar.dma_start(
                        out=y_d[P * vd : P * (vd + 1), 0:Fa], in_=xv[:, 0:Fa]
                    )
                nc.vector.tensor_scalar(
                    out=xv[:, Fa:F], in0=xv[:, Fa:F],
                    scalar1=sbv[:, 0:1], scalar2=sbv[:, 1:2],
                    op0=ALU.mult, op1=ALU.add,
                )
                nc.gpsimd.dma_start(out=y_d[P * vd : P * (vd + 1), Fa:F], in_=xv[:, Fa:F])

            if t >= 0:
                # funnel (sum, sumsq) through DVE so the PE matmul needs one wait
                part2 = ppool.tile([P, 2], f32, tag="part2")
                nc.vector.tensor_copy(part2[:], part[:])
                # fold Q partitions -> per-channel (mean, E[x^2])
                ch = ch_ps.tile([G, 2], f32)
                nc.tensor.matmul(ch[:], g4_f, part2[:], start=True, stop=True)
                st = sts[t] = stpool.tile([G, 8], f32, name="st")
                nc.vector.tensor_copy(st[:, 0:2], ch[:])
                nc.vector.tensor_mul(st[:, 2:3], st[:, 0:1], st[:, 0:1])
                nc.vector.tensor_sub(st[:, 3:4], st[:, 1:2], st[:, 2:3])  # var

            if u >= 0:
                stu = sts.pop(u)
                ud = u % NT
                nc.vector.reciprocal(stu[:, 7:8], stu[:, 6:7])
                nc.vector.tensor_mul(stu[:, 4:5], stu[:, 7:8], gb_all[:, ud, 0:1])
                nc.vector.tensor_mul(stu[:, 2:3], stu[:, 0:1], stu[:, 4:5])
                nc.vector.tensor_sub(stu[:, 5:6], gb_all[:, ud, 1:2], stu[:, 2:3])
                # broadcast per-channel (scale, bias) back to Q partitions each
                bc = bc_ps.tile([P, 2], f32)
                nc.tensor.matmul(bc[:], e4_f, stu[:, 4:6], start=True, stop=True)
                sb2 = sbs[u] = sbpool.tile([P, 2], f32, name="sb2")
                nc.vector.tensor_copy(sb2[:], bc[:])

    nc.compile()
    return nc


def make_consts(C_=C, HW_=HW, S_=S, G=DEFAULTS["G"], gammas=None, betas=None, sw=None):
    """Host-side packed constants tensor [128, NCOLS]."""
    Q = P // G
    o_g4, o_e4, o_gam, o_bet, o_sw, NCOLS = _const_layout(C_, S_, G)
    consts = np.zeros((P, NCOLS), np.float32)
    consts[np.arange(P), o_g4 + np.arange(P) // Q] = 1.0 / HW_
    consts[np.arange(P) // Q, o_e4 + np.arange(P)] = 1.0
    consts[0:S_, o_gam : o_gam + C_] = gammas
    consts[0:S_, o_bet : o_bet + C_] = betas
    consts[0:S_, o_sw] = sw
    return consts


def io_np_dtype(io_dtype=DEFAULTS["io_dtype"]):
    return mybir.dt.np(bf16 if io_dtype == "bf16" else f32)


def prep_x(x, G=DEFAULTS["G"], io_dtype=DEFAULTS["io_dtype"], B_=B, C_=C, HW_=HW):
    """Reshape full x [B,C,H,W] to per-core [C*Q, F] layout and cast to IO dtype."""
    Q = P // G
    return np.ascontiguousarray(
        np.asarray(x).reshape(B_, C_ * Q, HW_ // Q).astype(io_np_dtype(io_dtype))
    )


_CACHE = {}


def _get_nc():
    if "nc" not in _CACHE:
        _CACHE["nc"] = build_cin_program()
    return _CACHE["nc"]


def kernel(x, style_weights, gammas, betas, _trace=False):
    x = np.asarray(x, dtype=np.float32)
    style_weights = np.ascontiguousarray(np.asarray(style_weights, dtype=np.float32))
    gammas = np.ascontiguousarray(np.asarray(gammas, dtype=np.float32))
    betas = np.ascontiguousarray(np.asarray(betas, dtype=np.float32))

    G = DEFAULTS["G"]
    nc = _get_nc()

    xr = prep_x(x, G)
    in_maps = [
        {
            "x": xr[i],
            "consts": make_consts(C, HW, S, G, gammas, betas, style_weights[i]),
        }
        for i in range(N_CORES)
    ]
    res = run_bass_kernel_spmd(
        nc, in_maps, core_ids=list(range(N_CORES)), trace=_trace
    )
    y = np.stack(
        [
            res.results[i]["y"].astype(np.float32).reshape(C, H, W)
            for i in range(N_CORES)
        ],
        axis=0,
    )
    if _trace:
        return y, res
    return y
